# revision 1
# baseline (speedup 1.0000x reference)
"""DLRM dot-interaction + tril gather + concat kernel for Trainium2 (raw Bass).

features [B,27,128] f32, bottom_mlp_out [B,128] f32 ->
out [B, 479] = concat(bottom[b], tril(features[b] @ features[b].T)).

Data-parallel over 8 cores (B=65536 -> 8192/core). Per 128-sample megagroup:
  1. SP DMA: features chunk -> A [108, 4096] (natural layout).
  2. PE: 32 transposes (identity matmul) -> pt PSUM banks; DVE/ACT copy to
     XT [128, 3488] (d on partitions, 27 cols/sample + 32 pad cols).
  3. PE: 128 per-sample Gram matmuls (lhsT widened to 32 cols so all psum rows
     are written), col-tiled via tile_position into Gstack [128, 1024] PSUM:
     sample lb = 32*P + 16*k + b1 -> partitions [32P,32P+32), cols 512k+27b1.
  4. DVE: nested-AP copy un-interleaves Gstack -> GS2[p, 32j+c]; 32x32
     stream-transpose -> FLAT[b, 32j+i] = G_b[i,j] (sample-major).
  5. Pool: 26 strided copies gather strict lower triangle -> OUTT [128, 351].
  6. SP DMA: OUTT -> out rows [128:479]; ACT DMA ring: bottom -> out[:, 0:128]
     (DRAM->DRAM, no deps).

All synchronization is explicit (one semaphore wait per wait_ge instruction;
this walrus build rejects instructions carrying more than one embedded wait).
"""

import numpy as np

import concourse.bass as bass
import concourse.mybir as mybir

F = 27
D = 128
NPAIR = F * (F - 1) // 2  # 351
OUTW = D + NPAIR  # 479
MG = 128
N_CORES = 8
B_FULL = 65536
NB = B_FULL // N_CORES

FP32 = mybir.dt.float32


def tri(i):
    return i * (i - 1) // 2


def build_nc(nb=NB):
    assert nb % MG == 0
    n_mg = nb // MG
    nc = bass.Bass()
    feats = nc.dram_tensor("features", [nb, F, D], FP32, kind="ExternalInput")
    bottom = nc.dram_tensor("bottom_mlp_out", [nb, D], FP32, kind="ExternalInput")
    out = nc.dram_tensor("out", [nb, OUTW], FP32, kind="ExternalOutput")

    feats_flat = feats[:].rearrange("b f d -> (b f) d")  # [nb*27, 128]

    XTW = MG * F + 32  # 3488

    from contextlib import ExitStack

    with ExitStack() as ctx:
        sem = lambda n: ctx.enter_context(nc.semaphore(n))
        sb = lambda n, s: ctx.enter_context(nc.sbuf_tensor(n, s, FP32))
        ps = lambda n, s: ctx.enter_context(nc.psum_tensor(n, s, FP32))
        s_id, s_t, s_cv, s_ca, s_mm = (
            sem("s_id"), sem("s_t"), sem("s_cv"), sem("s_ca"), sem("s_mm"),
        )
        s_g2, s_tr, s_ga, s_bot, s_pad = (
            sem("s_g2"), sem("s_tr"), sem("s_ga"), sem("s_bot"), sem("s_pad"),
        )
        s_a = [sem("s_a0"), sem("s_a1")]
        s_do = [sem("s_do0"), sem("s_do1")]
        ones = sb("ones", [128, 128])
        ident = sb("ident", [128, 128])
        A = [sb("A0", [128, 3456]), sb("A1", [128, 3456])]
        XT = [sb("XT0", [128, XTW]), sb("XT1", [128, XTW])]
        GS2 = [sb("GS2_0", [128, 864]), sb("GS2_1", [128, 864])]
        FLAT = [sb("FLAT0", [128, 864]), sb("FLAT1", [128, 864])]
        OUTT = [sb("OUTT0", [128, NPAIR]), sb("OUTT1", [128, NPAIR])]
        pt = [ps(f"pt{i}", [128, 384]) for i in range(4)]
        gs = [ps("gs0", [128, 1024]), ps("gs1", [128, 1024])]
        block = ctx.enter_context(nc.Block())
        XT0, XT1 = XT

        # M1 bookkeeping: 27 transpose chunks of 128 flat-rows per megagroup,
        # grouped 3 per psum bank; banks rotate globally (sigma = 9g + R) over
        # the 4 pt tensors. Copies alternate DVE (R even, 5/mg) / ACT (R odd).
        def ndve(sigma):  # DVE copies with global index <= sigma
            if sigma < 0:
                return 0
            return 5 * (sigma // 9) + (sigma % 9 + 2) // 2

        def nact(sigma):
            if sigma < 0:
                return 0
            return 4 * (sigma // 9) + (sigma % 9 + 1) // 2

        def a_src(g):
            return feats_flat[3456 * g : 3456 * (g + 1), :].rearrange(
                "(c p) d -> p c d", p=128
            )

        @block.sync
        def _(sync):
            for g in range(n_mg):
                sl = g % 2
                if g >= 2:
                    sync.wait_ge(s_ga, g - 1)
                    sync.dma_start(
                        out[MG * (g - 2) : MG * (g - 1), D:OUTW], OUTT[g % 2][:]
                    ).then_inc(s_do[g % 2], 16)
                    sync.wait_ge(s_t, 9 * (g - 2) + 9)
                sync.dma_start(
                    A[sl][:].rearrange("p (q d) -> p q d", d=D), a_src(g)
                ).then_inc(s_a[sl], 16)
            for g in (n_mg - 2, n_mg - 1):
                if g < 0:
                    continue
                sync.wait_ge(s_ga, g + 1)
                sync.dma_start(
                    out[MG * g : MG * (g + 1), D:OUTW], OUTT[g % 2][:]
                ).then_inc(s_do[g % 2], 16)
            for sl in range(2):
                sync.wait_ge(s_do[sl], 16 * ((n_mg + 1 - sl) // 2))
            sync.wait_ge(s_bot, 16)

        @block.tensor
        def _(tensor):
            tensor.wait_ge(s_id, 2)
            tensor.wait_ge(s_pad, 2)
            for g in range(n_mg):
                sl = g % 2
                tensor.wait_ge(s_a[sl], 16 * (g // 2 + 1))
                # 27 transposes in 9 banks of 3, emitted in thirds
                for h in range(3):
                    sig = 9 * g + 3 * h - 2  # pt-bank WAR horizon
                    if ndve(sig) > 0:
                        tensor.wait_ge(s_cv, ndve(sig))
                    if nact(sig) > 0:
                        tensor.wait_ge(s_ca, nact(sig))
                    for R in range(3 * h, 3 * h + 3):
                        bank = (9 * g + R) % 4
                        for j in range(3):
                            c = 3 * R + j
                            ins = nc.tensor.transpose(
                                pt[bank][:, 128 * j : 128 * (j + 1)],
                                A[sl][:, 128 * c : 128 * (c + 1)],
                                ident[:],
                            )
                            if j == 2:
                                ins.then_inc(s_t, 1)
                # Gram matmuls, two halves of 64 samples
                if g >= 2:
                    tensor.wait_ge(s_g2, g - 1)
                for half in range(2):
                    tensor.wait_ge(s_cv, ndve(9 * g + 8) if half else ndve(9 * g + 4))
                    tensor.wait_ge(s_ca, nact(9 * g + 8) if half else nact(9 * g + 4))
                    for lb in range(64 * half, 64 * half + 64):
                        P, rem = divmod(lb, 32)
                        k, b1 = divmod(rem, 16)
                        c0 = 512 * k + F * b1
                        ins = nc.tensor.matmul(
                            gs[sl][32 * P : 32 * P + 32, c0 : c0 + F],
                            XT[sl][:, F * lb : F * lb + 32],
                            XT[sl][:, F * lb : F * lb + F],
                            start=True,
                            stop=True,
                            tile_position=(0, 32 * P),
                        )
                        if lb == 127:
                            ins.then_inc(s_mm, 1)

        @block.vector
        def _(vector):
            vector.memset(XT0[:, MG * F : XTW], 0.0).then_inc(s_pad, 1)
            vector.memset(XT1[:, MG * F : XTW], 0.0).then_inc(s_pad, 1)
            for g in range(n_mg):
                sl = g % 2
                for R in (0, 2, 4, 6, 8):
                    vector.wait_ge(s_t, 9 * g + R + 1)
                    if R == 0 and g >= 2:
                        vector.wait_ge(s_mm, g - 1)
                    vector.tensor_copy(
                        XT[sl][:, 384 * R : 384 * (R + 1)], pt[(9 * g + R) % 4][:]
                    ).then_inc(s_cv, 1)
                vector.wait_ge(s_mm, g + 1)
                in_ap = bass.AP(gs[sl], 0, [[1024, 128], [1, 27], [512, 2], [27, 16]])
                vector.tensor_copy(GS2[sl][:], in_ap).then_inc(s_g2, 1)
                if g >= 2:
                    vector.wait_ge(s_ga, g - 1)
                vector.wait_ge(s_g2, g + 1)
                vector.transpose(FLAT[sl][:], GS2[sl][:]).then_inc(s_tr, 1)

        @block.scalar
        def _(scalar):
            scalar.dma_start(out[:, 0:D], bottom[:, :]).then_inc(s_bot, 16)
            for g in range(n_mg):
                sl = g % 2
                first = True
                for R in (1, 3, 5, 7):
                    scalar.wait_ge(s_t, 9 * g + R + 1)
                    if first and g >= 2:
                        scalar.wait_ge(s_mm, g - 1)
                    first = False
                    scalar.copy(
                        XT[sl][:, 384 * R : 384 * (R + 1)], pt[(9 * g + R) % 4][:]
                    ).then_inc(s_ca, 1)

        @block.gpsimd
        def _(gpsimd):
            gpsimd.memset(ones[:], 1.0).then_inc(s_id, 1)
            gpsimd.wait_ge(s_id, 1)
            gpsimd.affine_select(
                ident[:],
                ones[:],
                pattern=[[1, 128]],
                compare_op=mybir.AluOpType.is_equal,
                fill=0.0,
                base=0,
                channel_multiplier=-1,
            ).then_inc(s_id, 1)
            for g in range(n_mg):
                sl = g % 2
                gpsimd.wait_ge(s_tr, g + 1)
                if g >= 2:
                    gpsimd.wait_ge(s_do[g % 2], 16 * (g // 2))
                for i in range(1, F):
                    src = bass.AP(FLAT[sl], i, [[864, 128], [32, i]])
                    ins = gpsimd.tensor_copy(OUTT[sl][:, tri(i) : tri(i) + i], src)
                    if i == F - 1:
                        ins.then_inc(s_ga, 1)

    return nc


_NC_CACHE = {}


def _get_nc(nb):
    if nb not in _NC_CACHE:
        _NC_CACHE[nb] = build_nc(nb)
    return _NC_CACHE[nb]


def kernel(features: np.ndarray, bottom_mlp_out: np.ndarray) -> np.ndarray:
    from concourse.bass_utils import run_bass_kernel_spmd

    B = features.shape[0]
    nb = B // N_CORES
    nc = _get_nc(nb)
    features = np.ascontiguousarray(features, dtype=np.float32)
    bottom_mlp_out = np.ascontiguousarray(bottom_mlp_out, dtype=np.float32)
    in_maps = [
        {
            "features": features[i * nb : (i + 1) * nb],
            "bottom_mlp_out": bottom_mlp_out[i * nb : (i + 1) * nb],
        }
        for i in range(N_CORES)
    ]
    res = run_bass_kernel_spmd(nc, in_maps, core_ids=list(range(N_CORES)))
    return np.concatenate([r["out"] for r in res.results], axis=0)



# revision 4
# speedup vs baseline: 152.8861x; 152.8861x over previous
"""DLRM dot-interaction kernel v5 for Trainium2 (raw Bass).

features [B,27,128] f32, bottom_mlp_out [B,128] f32 ->
out [B, 479] = concat(bottom[b], tril(features[b] @ features[b].T)).

Per 128-sample megagroup (8192 samples/core, data-parallel over 8 cores):
  1. SP DMA: features chunk -> A (c p)-layout (chunk c = flat rows
     [128c,128c+128), partition = row-within-chunk); viewed as bf16: the
     HIGH u16 halfword of each f32 is its bf16 truncation.
     Also: bottom[group] -> OUTC[gg%6][:, 0:128].
  2. PE: 27 transposes of the ODD u16 columns (= bf16-truncated values) at
     1 cyc/row -> pt PSUM (bf16); DVE(5)/ACT(4) 16-bit copies to XT
     [128, 3488] bf16, SAMPLE-major: XT[d, 27*lb + f] = bf16(X_lb[f, d]).
  3. PE: 128 per-sample Gram matmuls, CONTIGUOUS 32-col lhsT / 27-col rhs
     (strided-column APs measured 3.3x slower), tile_position col-tiling
     into gs [128,1024] f32 PSUM: sample lb = 32P+16k+b1 -> partitions
     [32P,32P+32), cols 512k+27b1.
  4. ACT: nested-AP copy un-interleaves gs -> GS2; DVE: 32x32
     stream-transpose -> FLAT[b, 32j+i] = G_b[i,j] (sample-major), emitted
     AFTER the next group's copies so it never blocks the PE feed chain.
  5. Pool: 26 strided copies gather the strict lower triangle into
     OUTC[gg%6][:, 128+tri(i) : ...].
  6. SP DMA (lagged 4 groups): OUTC -> out rows as single 1916B
     descriptors per sample (bottom rides along).

iters>1 unrolls the pipeline over the same DRAM buffers (timing variant).
"""

import numpy as np

import concourse.bass as bass
import concourse.mybir as mybir

F = 27
D = 128
NPAIR = F * (F - 1) // 2  # 351
OUTW = D + NPAIR  # 479
MG = 128
N_CORES = 8
B_FULL = 65536
NB = B_FULL // N_CORES

FP32 = mybir.dt.float32
BF16 = mybir.dt.bfloat16

NO = 6   # OUTC slots
LAG = 4  # out-DMA lag (groups)
XTW = MG * F + 32  # 3488


def tri(i):
    return i * (i - 1) // 2


def build_nc(nb=NB, iters=1):
    assert nb % MG == 0
    n_mg = nb // MG
    T = iters * n_mg
    nc = bass.Bass()
    feats = nc.dram_tensor("features", [nb, F, D], FP32, kind="ExternalInput")
    bottom = nc.dram_tensor("bottom_mlp_out", [nb, D], FP32, kind="ExternalInput")
    out = nc.dram_tensor("out", [nb, OUTW], FP32, kind="ExternalOutput")

    import ml_dtypes

    ident_bf = nc.inline_tensor(
        np.eye(128, dtype=np.float32).astype(ml_dtypes.bfloat16), name="ident_bf"
    )

    feats_flat = feats[:].rearrange("b f d -> (b f) d")  # [nb*27, 128]

    from contextlib import ExitStack

    with ExitStack() as ctx:
        sem = lambda n: ctx.enter_context(nc.semaphore(n))
        sb = lambda n, s, dt=FP32: ctx.enter_context(nc.sbuf_tensor(n, s, dt))
        ps = lambda n, s, dt=FP32: ctx.enter_context(nc.psum_tensor(n, s, dt))
        s_id, s_t, s_cv, s_ca, s_mm = (
            sem("s_id"), sem("s_t"), sem("s_cv"), sem("s_ca"), sem("s_mm"),
        )
        s_g2, s_tr, s_ga, s_bot, s_pad = (
            sem("s_g2"), sem("s_tr"), sem("s_ga"), sem("s_bot"), sem("s_pad"),
        )
        s_a = [sem("s_a0"), sem("s_a1")]
        s_do = [sem(f"s_do{i}") for i in range(NO)]
        identb = sb("identb", [128, 128], BF16)
        A = [sb("A0", [128, F * D]), sb("A1", [128, F * D])]
        XT = [sb("XT0", [128, XTW], BF16), sb("XT1", [128, XTW], BF16)]
        GS2 = [sb("GS2_0", [128, 864]), sb("GS2_1", [128, 864])]
        FLAT = [sb("FLAT0", [128, 864]), sb("FLAT1", [128, 864])]
        OUTC = [sb(f"OUTC{i}", [128, OUTW]) for i in range(NO)]
        pt = [ps(f"pt{i}", [128, 384], BF16) for i in range(4)]
        gs = [ps("gs0", [128, 1024]), ps("gs1", [128, 1024])]
        block = ctx.enter_context(nc.Block())
        XT0, XT1 = XT

        # 27 transpose chunks per group, 3 per psum bank; banks rotate
        # globally (sigma = 9g + R). Copies: R even -> DVE (5/group),
        # R odd -> ACT (4/group).
        def ndve(sigma):
            if sigma < 0:
                return 0
            return 5 * (sigma // 9) + (sigma % 9 + 2) // 2

        def nact(sigma):
            if sigma < 0:
                return 0
            return 4 * (sigma // 9) + (sigma % 9 + 1) // 2

        def a_src(g):
            return feats_flat[3456 * g : 3456 * (g + 1), :].rearrange(
                "(c p) d -> p c d", p=128
            )

        def outc_uses(i, upto):
            if upto <= i:
                return 0
            return (upto - i + NO - 1) // NO

        @block.sync
        def _(sync):
            sync.dma_start(identb[:], ident_bf[:]).then_inc(s_id, 16)
            for gg in range(T):
                g = gg % n_mg
                sl = gg % 2
                so = gg % NO
                if gg >= 2:
                    sync.wait_ge(s_t, 9 * (gg - 1))
                sync.dma_start(
                    A[sl][:].rearrange("p (q d) -> p q d", d=D), a_src(g)
                ).then_inc(s_a[sl], 16)
                if gg >= NO:
                    sync.wait_ge(s_do[so], 16 * (gg // NO))
                sync.dma_start(
                    OUTC[so][:, 0:D], bottom[MG * g : MG * (g + 1), :]
                ).then_inc(s_bot, 16)
                if gg >= LAG:
                    gp = (gg - LAG) % n_mg
                    sync.wait_ge(s_ga, gg - LAG + 1)
                    sync.wait_ge(s_bot, 16 * (gg - LAG + 1))
                    sync.dma_start(
                        out[MG * gp : MG * (gp + 1), :], OUTC[(gg - LAG) % NO][:]
                    ).then_inc(s_do[(gg - LAG) % NO], 16)
            for k in range(max(0, T - LAG), T):
                gp = k % n_mg
                sync.wait_ge(s_ga, k + 1)
                sync.wait_ge(s_bot, 16 * (k + 1))
                sync.dma_start(
                    out[MG * gp : MG * (gp + 1), :], OUTC[k % NO][:]
                ).then_inc(s_do[k % NO], 16)
            for i in range(NO):
                n = outc_uses(i, T)
                if n:
                    sync.wait_ge(s_do[i], 16 * n)

        @block.tensor
        def _(tensor):
            tensor.wait_ge(s_id, 16)
            tensor.wait_ge(s_pad, 2)
            for gg in range(T):
                sl = gg % 2
                tensor.wait_ge(s_a[sl], 16 * (gg // 2 + 1))
                for h in range(3):
                    sig = 9 * gg + 3 * h - 2  # pt-bank WAR horizon
                    if ndve(sig) > 0:
                        tensor.wait_ge(s_cv, ndve(sig))
                    if nact(sig) > 0:
                        tensor.wait_ge(s_ca, nact(sig))
                    for R in range(3 * h, 3 * h + 3):
                        bank = (9 * gg + R) % 4
                        for j in range(3):
                            c = 3 * R + j
                            # odd u16 columns of f32 chunk = bf16 truncation
                            in_ap = bass.AP(
                                A[sl].bitcast(BF16), 256 * c + 1, [[6912, 128], [2, 128]]
                            )
                            ins = nc.tensor.matmul(
                                pt[bank][:, 128 * j : 128 * (j + 1)],
                                in_ap,
                                identb[:],
                                is_transpose=True,
                            )
                            if j == 2:
                                ins.then_inc(s_t, 1)
                # Gram matmuls (bf16, contiguous sample-major), two halves
                if gg >= 2:
                    tensor.wait_ge(s_g2, gg - 1)
                for half in range(2):
                    tensor.wait_ge(s_cv, ndve(9 * gg + 8) if half else ndve(9 * gg + 4))
                    tensor.wait_ge(s_ca, nact(9 * gg + 8) if half else nact(9 * gg + 4))
                    for lb in range(64 * half, 64 * half + 64):
                        P, rem = divmod(lb, 32)
                        k, b1 = divmod(rem, 16)
                        c0 = 512 * k + F * b1
                        ins = nc.tensor.matmul(
                            gs[sl][32 * P : 32 * P + 32, c0 : c0 + F],
                            XT[sl][:, F * lb : F * lb + 32],
                            XT[sl][:, F * lb : F * lb + F],
                            start=True,
                            stop=True,
                            tile_position=(0, 32 * P),
                        )
                        if lb == 127:
                            ins.then_inc(s_mm, 1)

        @block.vector
        def _(vector):
            vector.memset(XT0[:, MG * F : XTW], 0.0).then_inc(s_pad, 1)
            vector.memset(XT1[:, MG * F : XTW], 0.0).then_inc(s_pad, 1)

            for gg in range(T):
                sl = gg % 2
                for R in (0, 2, 4, 6, 8):
                    vector.wait_ge(s_t, 9 * gg + R + 1)
                    if R == 0 and gg >= 2:
                        vector.wait_ge(s_mm, gg - 1)
                    vector.tensor_copy(
                        XT[sl][:, 384 * R : 384 * (R + 1)], pt[(9 * gg + R) % 4][:]
                    ).then_inc(s_cv, 1)
                # 32x32 stream-transpose GS2 -> FLAT (sample-major)
                if gg >= 2:
                    vector.wait_ge(s_ga, gg - 1)
                vector.wait_ge(s_g2, gg + 1)
                vector.transpose(FLAT[sl][:], GS2[sl][:]).then_inc(s_tr, 1)

        @block.scalar
        def _(scalar):
            for gg in range(T):
                sl = gg % 2
                first = True
                for R in (1, 3, 5, 7):
                    scalar.wait_ge(s_t, 9 * gg + R + 1)
                    if first and gg >= 2:
                        scalar.wait_ge(s_mm, gg - 1)
                    first = False
                    scalar.copy(
                        XT[sl][:, 384 * R : 384 * (R + 1)], pt[(9 * gg + R) % 4][:]
                    ).then_inc(s_ca, 1)
                # un-interleave gs -> GS2 (WAR: stream-transpose of gg-2 done)
                scalar.wait_ge(s_mm, gg + 1)
                if gg >= 2:
                    scalar.wait_ge(s_tr, gg - 1)
                in_ap = bass.AP(gs[sl], 0, [[1024, 128], [1, 27], [512, 2], [27, 16]])
                scalar.copy(GS2[sl][:], in_ap).then_inc(s_g2, 1)

        @block.gpsimd
        def _(gpsimd):
            for gg in range(T):
                sl = gg % 2
                so = gg % NO
                gpsimd.wait_ge(s_tr, gg + 1)
                if gg >= NO:
                    gpsimd.wait_ge(s_do[so], 16 * (gg // NO))
                for i in range(1, F):
                    src = bass.AP(FLAT[sl], i, [[864, 128], [32, i]])
                    ins = gpsimd.tensor_copy(
                        OUTC[so][:, D + tri(i) : D + tri(i) + i], src
                    )
                    if i == F - 1:
                        ins.then_inc(s_ga, 1)

    return nc


_NC_CACHE = {}


def _get_nc(nb):
    if nb not in _NC_CACHE:
        _NC_CACHE[nb] = build_nc(nb)
    return _NC_CACHE[nb]


def kernel(features: np.ndarray, bottom_mlp_out: np.ndarray) -> np.ndarray:
    from concourse.bass_utils import run_bass_kernel_spmd

    B = features.shape[0]
    nb = B // N_CORES
    nc = _get_nc(nb)
    features = np.ascontiguousarray(features, dtype=np.float32)
    bottom_mlp_out = np.ascontiguousarray(bottom_mlp_out, dtype=np.float32)
    in_maps = [
        {
            "features": features[i * nb : (i + 1) * nb],
            "bottom_mlp_out": bottom_mlp_out[i * nb : (i + 1) * nb],
        }
        for i in range(N_CORES)
    ]
    res = run_bass_kernel_spmd(nc, in_maps, core_ids=list(range(N_CORES)))
    return np.concatenate([r["out"] for r in res.results], axis=0)
